# revision 8
# baseline (speedup 1.0000x reference)
"""ARIMA(4,1,2)+exog Trainium2 kernel, data-parallel over 8 NeuronCores.

Per batch row (derived from the reference):
  m=4; steps = T-1-m
  e_i = sum_{j=0..5} g_j x[i+j] - feat_i - bias       (feat_i = features[i+4] . w)
  res'_i = e_i - c1 res'_{i-1} - c0 res'_{i-2}  (zero IC; c0,c1 = ma_coef)
  out[0] = x[0]; out[i+1] = x0 - x4 + x[i+5] - cumsum(res')_i - c1 e0 V_i
The IIR 1/A(z) is an exact-to-f32 FIR via root-doubling:
  1/A(z) = Abar(z) * Bbar(z^2) / C(z^4); C's roots ~ rho^4 -> 5 taps:
  v1 = D(z^4) e;  v2 = v1 - beta v1(-2) + gam v1(-4);
  res = v2 - c1 v2(-1) + c0 v2(-2)

Device layout (per core, 32 rows): partitions p = 32*q + r fold each row's
timeline into NQ=4 quarters of TQ=4096. Features arrive host-transposed
(r, f, t) so the F-reduction is a TensorE matmul with block-diagonal weights.
"""

import numpy as np

import concourse.bass as bass
import concourse.bacc as bacc
import concourse.mybir as mybir
import concourse.tile as tile
from concourse.bass_utils import run_bass_kernel_spmd

FP = mybir.dt.float32
OP = mybir.AluOpType

B, T, F = 256, 16384, 32
NCORES = 8
R = B // NCORES            # 32 rows per core
M_LAG = 4
STEPS = T - 1 - M_LAG      # 16379

NQ = 4                     # fold factor: partition p = 32*q + r
TQ = T // NQ               # 4096
CH = 1024                  # chunk length
NCH = TQ // CH             # 4
MMN = 512                  # matmul free dim (one PSUM bank)
NMM = CH // MMN            # 2
PATCH = 32                 # quarter-head patch width (> FIR span 22)
XPAD = T + 16
TPAD = T + 8
OPAD = T + 4

LAST_RESULT = None


def _fir_taps(c0, c1):
    beta = 2.0 * c0 - c1 * c1
    gam = c0 * c0
    p = 2.0 * gam - beta * beta
    q = gam * gam
    d = [1.0, -p, p * p - q, -p ** 3 + 2 * p * q, p ** 4 - 3 * p * p * q + q * q]
    return beta, gam, d


def build_nc(ar, c0, c1, bias):
    g = [0.0] * 6
    g[5] += 1.0
    g[4] -= 1.0
    for k in range(4):
        g[k] += ar[k]
        g[k + 1] -= ar[k]
    beta, gam, dtap = _fir_taps(c0, c1)

    nc = bacc.Bacc(None, target_bir_lowering=False)
    xp_d = nc.declare_dram_parameter("xp", [R, XPAD], FP, isOutput=False)
    ft_d = nc.declare_dram_parameter("ft", [8, 128, NQ, TQ], FP, isOutput=False)
    w_d = nc.declare_dram_parameter("wmat", [8, 128, 32], FP, isOutput=False)
    v_d = nc.declare_dram_parameter("vrep", [128, TQ], FP, isOutput=False)
    qm_d = nc.declare_dram_parameter("qmask", [128, 128], FP, isOutput=False)
    out_d = nc.declare_dram_parameter("out", [R, OPAD], FP, isOutput=True)

    def stt(out, in0, scl, in1):
        nc.vector.scalar_tensor_tensor(out, in0, float(scl), in1, OP.mult, OP.add)

    with tile.TileContext(nc) as tc:
        with (
            tc.tile_pool(name="fixed", bufs=1) as fixed,
            tc.tile_pool(name="gtiles", bufs=3) as gpool,
            tc.tile_pool(name="scr", bufs=1) as scr,
            tc.tile_pool(name="scr2", bufs=1) as scr2,
            tc.tile_pool(name="outp", bufs=2) as outp,
            tc.tile_pool(name="small", bufs=1) as small,
            tc.tile_pool(name="psum", bufs=3, space=bass.MemorySpace.PSUM) as psum,
            tc.tile_pool(name="psoff", bufs=1, space=bass.MemorySpace.PSUM) as psoff,
        ):
            x_ext = fixed.tile([128, TQ + 8], FP)
            feat = fixed.tile([128, TQ], FP)
            e_b = fixed.tile([128, TQ], FP)
            res = fixed.tile([128, TQ], FP)
            s_b = fixed.tile([128, TQ], FP)
            vrep = fixed.tile([128, TQ], FP)
            wsb = fixed.tile([128, 8 * 32], FP)
            qmask = fixed.tile([128, 128], FP)

            # x_ext[32q+r, j] = xp[r, TQ*q + j]
            nc.sync.dma_start(
                x_ext[:], bass.AP(xp_d, 0, [[TQ, NQ], [XPAD, R], [1, TQ + 8]])
            )
            # wsb[p, 32g+m] = wmat[g, p, m]
            nc.sync.dma_start(
                wsb[:], bass.AP(w_d, 0, [[32, 128], [128 * 32, 8], [1, 32]])
            )
            nc.sync.dma_start(vrep[:], v_d[:, :])
            nc.sync.dma_start(qmask[:], qm_d[:, :])

            ones = small.tile([128, CH], FP)
            nc.vector.memset(ones[:], 1.0)

            e0_bc = small.tile([128, 1], FP)
            cpp = small.tile([128, 1], FP)
            c2 = small.tile([128, 1], FP)
            off_sb = small.tile([128, 1], FP)

            # cpp = x0 - x4 per row, broadcast to all quarters
            nc.vector.tensor_tensor(
                cpp[0:R, :], x_ext[0:R, 0:1], x_ext[0:R, 4:5], OP.subtract
            )
            for q in range(1, NQ):
                nc.sync.dma_start(cpp[R * q:R * (q + 1), :], cpp[0:R, :])

            # ---------------- streamed main loop ----------------
            for c in range(NCH):
                c0i = c * CH
                pt = psum.tile([128, CH], FP, tag="pt")
                for gi in range(8):
                    gt = gpool.tile([128, NQ, CH], FP, tag="gt")
                    nc.sync.dma_start(
                        gt[:],
                        bass.AP(
                            ft_d,
                            gi * 128 * NQ * TQ + c0i,
                            [[NQ * TQ, 128], [TQ, NQ], [1, CH]],
                        ),
                    )
                    for q in range(NQ):
                        for h in range(NMM):
                            nc.tensor.matmul(
                                pt[R * q:R * (q + 1), h * MMN:(h + 1) * MMN],
                                wsb[:, 32 * gi:32 * (gi + 1)],
                                gt[:, q, h * MMN:(h + 1) * MMN],
                                start=(gi == 0),
                                stop=(gi == 7),
                                tile_position=(0, R * q),
                                skip_group_check=True,
                            )
                nc.scalar.copy(feat[:, c0i:c0i + CH], pt[:])

                # ---- e assembly: e = sum_j g_j x(+j) - feat - bias ----
                ea = scr.tile([128, CH], FP, tag="ea")
                eb2 = scr.tile([128, CH], FP, tag="eb2")
                nc.vector.tensor_scalar(
                    ea[:], x_ext[:, c0i:c0i + CH],
                    float(g[0]), float(-bias), OP.mult, OP.add,
                )
                src, dst = ea, eb2
                for j in range(1, 6):
                    stt(dst[:], x_ext[:, c0i + j:c0i + j + CH], g[j], src[:])
                    src, dst = dst, src
                stt(e_b[:, c0i:c0i + CH], feat[:, c0i:c0i + CH], -1.0, src[:])
                if c == 0:
                    for q in range(NQ):
                        nc.sync.dma_start(
                            e0_bc[R * q:R * (q + 1), :], e_b[0:R, 0:1]
                        )

                # ---- FIR ----
                lo2 = max(0, c0i - PATCH)
                ex2 = c0i + CH - lo2         # CH (c=0) or CH+PATCH
                va = scr2.tile([128, PATCH + CH], FP, tag="va")
                vb = scr2.tile([128, PATCH + CH], FP, tag="vb")
                vc = scr2.tile([128, PATCH + CH], FP, tag="vc")

                # v1 = e + sum_k d_k e(-4k); in0 always absolute e_b
                cur, oth = va, vb
                prev_is_e = True
                for k in (1, 2, 3, 4):
                    sh = 4 * k
                    st = lo2 - sh
                    ofs = max(0, -st)
                    in1 = e_b[:, lo2 + ofs:lo2 + ex2] if prev_is_e \
                        else cur[:, ofs:ex2]
                    stt(oth[:, ofs:ex2], e_b[:, st + ofs:st + ex2], dtap[k], in1)
                    if ofs > 0:
                        head = e_b[:, lo2:lo2 + ofs] if prev_is_e \
                            else cur[:, 0:ofs]
                        nc.vector.tensor_copy(oth[:, 0:ofs], head)
                    cur, oth = oth, cur
                    prev_is_e = False
                v1 = cur                       # = vb (4 swaps from va start)
                t1, v2 = oth, vc
                # v2 = v1 - beta v1(-2) + gam v1(-4)
                stt(t1[:, 2:ex2], v1[:, 0:ex2 - 2], -beta, v1[:, 2:ex2])
                nc.vector.tensor_copy(t1[:, 0:2], v1[:, 0:2])
                stt(v2[:, 4:ex2], v1[:, 0:ex2 - 4], gam, t1[:, 4:ex2])
                nc.vector.tensor_copy(v2[:, 0:4], t1[:, 0:4])
                # res = v2 - c1 v2(-1) + c0 v2(-2)
                r1 = v1                       # v1 dead, reuse
                stt(r1[:, 1:ex2], v2[:, 0:ex2 - 1], -c1, v2[:, 1:ex2])
                nc.vector.tensor_copy(r1[:, 0:1], v2[:, 0:1])
                if c == 0:
                    stt(res[:, 2:CH], v2[:, 0:CH - 2], c0, r1[:, 2:CH])
                    nc.vector.tensor_copy(res[:, 0:2], r1[:, 0:2])
                else:
                    stt(
                        res[:, c0i:c0i + CH], v2[:, ex2 - CH - 2:ex2 - 2],
                        c0, r1[:, ex2 - CH:ex2],
                    )

            # ---------------- quarter-head patch ----------------
            # strip col i <-> t = TQ*q - PATCH + i; outputs [PATCH, 2*PATCH)
            W2 = 2 * PATCH
            pb = small.tile([128, W2], FP)
            pa = small.tile([128, W2], FP)
            pc = small.tile([128, W2], FP)
            pdd = small.tile([128, W2], FP)
            nc.vector.memset(pb[0:R, 0:PATCH], 0.0)
            nc.sync.dma_start(pb[R:128, 0:PATCH], e_b[0:128 - R, TQ - PATCH:TQ])
            nc.vector.tensor_copy(pb[:, PATCH:W2], e_b[:, 0:PATCH])
            cur, oth = pa, pc
            prev_is_e = True
            for k in (1, 2, 3, 4):
                sh = 4 * k
                in1 = pb[:, sh:W2] if prev_is_e else cur[:, sh:W2]
                stt(oth[:, sh:W2], pb[:, 0:W2 - sh], dtap[k], in1)
                nc.vector.tensor_copy(
                    oth[:, 0:sh], pb[:, 0:sh] if prev_is_e else cur[:, 0:sh]
                )
                cur, oth = oth, cur
                prev_is_e = False
            v1p = cur
            t1p, v2p = oth, pdd
            stt(t1p[:, 2:W2], v1p[:, 0:W2 - 2], -beta, v1p[:, 2:W2])
            nc.vector.tensor_copy(t1p[:, 0:2], v1p[:, 0:2])
            stt(v2p[:, 4:W2], v1p[:, 0:W2 - 4], gam, t1p[:, 4:W2])
            nc.vector.tensor_copy(v2p[:, 0:4], t1p[:, 0:4])
            r1p = v1p
            stt(r1p[:, 1:W2], v2p[:, 0:W2 - 1], -c1, v2p[:, 1:W2])
            stt(
                res[:, 0:PATCH], v2p[:, PATCH - 2:W2 - 2],
                c0, r1p[:, PATCH:W2],
            )

            # ---------------- cumsum + offsets + output ----------------
            for c in range(NCH):
                c0i = c * CH
                init = 0.0 if c == 0 else s_b[:, c0i - 1:c0i]
                nc.vector.tensor_tensor_scan(
                    s_b[:, c0i:c0i + CH], ones[:], res[:, c0i:c0i + CH],
                    init, OP.mult, OP.add,
                )

            po = psoff.tile([128, 1], FP)
            nc.tensor.matmul(
                po[:], qmask[:], s_b[:, TQ - 1:TQ], start=True, stop=True
            )
            nc.scalar.copy(off_sb[:], po[:])
            nc.vector.tensor_tensor(c2[:], cpp[:], off_sb[:], OP.subtract)

            for c in range(NCH):
                c0i = c * CH
                ot = outp.tile([128, CH], FP, tag="ot")
                o2 = outp.tile([128, CH], FP, tag="o2")
                nc.vector.tensor_scalar(
                    ot[:], x_ext[:, c0i + 5:c0i + 5 + CH],
                    c2[:], None, OP.add,
                )
                stt(o2[:], s_b[:, c0i:c0i + CH], -1.0, ot[:])
                nc.vector.scalar_tensor_tensor(
                    ot[:], vrep[:, c0i:c0i + CH], e0_bc[:], o2[:],
                    OP.mult, OP.add,
                )
                nc.sync.dma_start(
                    bass.AP(out_d, 1 + c0i, [[TQ, NQ], [OPAD, R], [1, CH]]),
                    ot[:],
                )
            nc.sync.dma_start(
                bass.AP(out_d, 0, [[OPAD, R], [1, 1]]), x_ext[0:R, 0:1]
            )

    nc.compile()
    return nc


def _host_prep(ma_coef, feature_weights):
    c0, c1 = float(ma_coef[0]), float(ma_coef[1])
    w = np.asarray(feature_weights, np.float64)

    v = np.zeros(T, np.float64)
    if STEPS > 1:
        v[1] = 1.0
        for j in range(2, STEPS):
            v[j] = -c1 * v[j - 1] - c0 * v[j - 2]
    V = np.cumsum(v)
    V[STEPS:] = 0.0
    vq = (-c1 * V).astype(np.float32).reshape(NQ, TQ)
    vrep = np.ascontiguousarray(np.repeat(vq, R, axis=0)).astype(np.float32)
    # vrep[32q+r, j] = -c1*V[TQ*q+j]

    wmat = np.zeros((8, 128, 32), np.float32)
    for gi in range(8):
        for r in range(32):
            for fp in range(4):
                wmat[gi, 4 * r + fp, r] = np.float32(w[4 * gi + fp])

    qmask = np.zeros((128, 128), np.float32)
    for pq in range(NQ):
        for mq in range(NQ):
            if pq < mq:
                for r in range(R):
                    qmask[R * pq + r, R * mq + r] = 1.0
    return c0, c1, vrep, wmat, qmask


def kernel(x, features, ar_coef, ma_coef, feature_weights, bias):
    global LAST_RESULT
    x = np.ascontiguousarray(np.asarray(x, np.float32))
    features = np.ascontiguousarray(np.asarray(features, np.float32))
    ar = [float(a) for a in np.asarray(ar_coef)]
    bi = float(np.asarray(bias).reshape(-1)[0])
    c0, c1, vrep, wmat, qmask = _host_prep(ma_coef, feature_weights)

    nc = build_nc(ar, c0, c1, bi)

    in_maps = []
    for ci in range(NCORES):
        rs = slice(ci * R, (ci + 1) * R)
        xp = np.zeros((R, XPAD), np.float32)
        xp[:, :T] = x[rs]
        tmp = np.zeros((R, F, T), np.float32)
        tmp[:, :, : T - M_LAG] = features[rs, M_LAG:, :].transpose(0, 2, 1)
        ft = np.ascontiguousarray(
            tmp.reshape(R, 8, 4, NQ, TQ).transpose(1, 0, 2, 3, 4)
        ).reshape(8, 128, NQ, TQ)
        in_maps.append(
            {"xp": xp, "ft": ft, "wmat": wmat, "vrep": vrep, "qmask": qmask}
        )

    r = run_bass_kernel_spmd(nc, in_maps, core_ids=list(range(NCORES)))
    LAST_RESULT = r
    outs = [np.asarray(r.results[ci]["out"])[:, : STEPS + 1] for ci in range(NCORES)]
    return np.concatenate(outs, axis=0).astype(np.float32)
